# revision 5
# baseline (speedup 1.0000x reference)
"""Trainium2 Bass kernel for nn_MultiHeadDecoder (sparse attention decoder).

Math (reference, B=1, N=50000, D=512):
    concat    = W_context[0] @ [l; context]                  (1, D)
    g_context = W_graph[0]   @ [g; concat]                   (1, D)
    Q         = g_context @ W_query                          (1, D)
    K         = q @ W_key                                    (N, D)
    compat    = 10 * tanh(norm * Q @ K^T), masked -> -inf    (N,)
    outputs: q[argmax], softmax[argmax], log_softmax[argmax], concat, mask, argmax

Key algebraic optimization: scores = (q @ W_key) @ Q^T == q @ (W_key @ Q^T).
W_key @ Q^T is a tiny (D,D)@(D,) matvec done on host, so the device never
materializes K -- it streams q once and does a 50000x512 matvec + tanh +
masked softmax reductions. This is DMA-bound, not GEMM-bound.

Sharding: node dimension split across 8 cores, 6272 nodes/core (49 tiles of
128). Each core returns a per-partition (max, sum-exp, argmax) triple
([128,3] f32); the host does the final 1024-way combine (exact, tiny).
"""

import math

import numpy as np

import concourse.bass as bass
import concourse.tile as tile
from concourse import bacc, mybir
from concourse.bass_utils import run_bass_kernel_spmd

N_CORES = 8
N = 50000
D = 512
P = 128                      # SBUF partitions
NT = 49                      # node tiles per core
NS = P * NT                  # 6272 nodes per core
G = 7                        # node tiles per DMA chunk
N_PAD = N_CORES * NS         # 50176
NORM = 1.0 / math.sqrt(D)
MASK_NEG = -1000.0           # additive mask; scores are in [-10, 10]
BIG_IDX = 1.0e30

_prog_cache = {}


def _build_program():
    """One SPMD Bass program: scores for a 6272-node shard + reductions."""
    f32 = mybir.dt.float32
    nc = bacc.Bacc("TRN2", target_bir_lowering=False)

    qs = nc.dram_tensor("qs", [NS, D], f32, kind="ExternalInput")
    vb = nc.dram_tensor("vb", [1, D], f32, kind="ExternalInput")
    madd = nc.dram_tensor("madd", [P, NT], f32, kind="ExternalInput")
    mbin = nc.dram_tensor("mbin", [P, NT], f32, kind="ExternalInput")
    idxc = nc.dram_tensor("idxc", [P, NT], f32, kind="ExternalInput")
    stats = nc.dram_tensor("stats", [P, 3], f32, kind="ExternalOutput")

    # [NS, D] viewed as [P, NT, D]: column t of partition p is node t*128+p
    qsv = qs[:].rearrange("(t p) d -> p t d", p=P)

    with tile.TileContext(nc) as tc:
        with (
            tc.tile_pool(name="const", bufs=1) as constp,
            tc.tile_pool(name="qp", bufs=3) as qp,
            tc.tile_pool(name="work", bufs=2) as work,
            tc.tile_pool(name="acc", bufs=1) as accp,
        ):
            vbt = constp.tile([P, D], f32)
            nc.sync.dma_start(out=vbt[:], in_=vb[:].to_broadcast([P, D]))
            maddt = constp.tile([P, NT], f32)
            nc.sync.dma_start(out=maddt[:], in_=madd[:])
            mbint = constp.tile([P, NT], f32)
            nc.sync.dma_start(out=mbint[:], in_=mbin[:])
            idxt = constp.tile([P, NT], f32)
            nc.sync.dma_start(out=idxt[:], in_=idxc[:])
            bigt = constp.tile([P, NT], f32)
            nc.vector.memset(bigt[:], BIG_IDX)

            scores = accp.tile([P, NT], f32)

            for o in range(NT // G):
                qt = qp.tile([P, G, D], f32)
                nc.sync.dma_start(out=qt[:], in_=qsv[:, o * G : (o + 1) * G, :])
                for gi in range(G):
                    t = o * G + gi
                    prod = work.tile([P, D], f32)
                    # out = (q * 1.0) * v, accum_out = row-sum(out) = q . v
                    # (tensor_tensor_reduce dies at runtime on this stack;
                    # scalar_tensor_tensor's fused sum works)
                    nc.vector.scalar_tensor_tensor(
                        out=prod[:],
                        in0=qt[:, gi, :],
                        scalar=1.0,
                        in1=vbt[:],
                        op0=mybir.AluOpType.mult,
                        op1=mybir.AluOpType.mult,
                        accum_out=scores[:, t : t + 1],
                    )

            # tanh_t = tanh(norm * s);  cm = 10*tanh_t + madd (masked score)
            tanh_t = accp.tile([P, NT], f32)
            nc.scalar.activation(
                out=tanh_t[:], in_=scores[:],
                func=mybir.ActivationFunctionType.Tanh, scale=float(NORM),
            )
            cm = accp.tile([P, NT], f32)
            nc.vector.scalar_tensor_tensor(
                out=cm[:], in0=tanh_t[:], scalar=10.0, in1=maddt[:],
                op0=mybir.AluOpType.mult, op1=mybir.AluOpType.add,
            )

            st = accp.tile([P, 3], f32)
            # per-partition max of masked scores
            nc.vector.reduce_max(
                out=st[:, 0:1], in_=cm[:], axis=mybir.AxisListType.X
            )
            # e = exp(10*tanh_t); masked sum via e * mbin with fused row-sum
            e_t = accp.tile([P, NT], f32)
            nc.scalar.activation(
                out=e_t[:], in_=tanh_t[:],
                func=mybir.ActivationFunctionType.Exp, scale=10.0,
            )
            escr = accp.tile([P, NT], f32)
            nc.vector.scalar_tensor_tensor(
                out=escr[:], in0=e_t[:], scalar=1.0, in1=mbint[:],
                op0=mybir.AluOpType.mult, op1=mybir.AluOpType.mult,
                accum_out=st[:, 1:2],
            )
            # argmax: first (lowest local index) column hitting the row max
            # (CopyPredicated requires an integer mask dtype)
            iseq = accp.tile([P, NT], mybir.dt.int32)
            nc.vector.tensor_scalar(
                out=iseq[:], in0=cm[:], scalar1=st[:, 0:1], scalar2=None,
                op0=mybir.AluOpType.is_ge,
            )
            idxsel = accp.tile([P, NT], f32)
            nc.vector.select(idxsel[:], iseq[:], idxt[:], bigt[:])
            nc.vector.tensor_reduce(
                out=st[:, 2:3], in_=idxsel[:],
                op=mybir.AluOpType.min, axis=mybir.AxisListType.X,
            )

            nc.sync.dma_start(out=stats[:], in_=st[:])

    nc.compile()
    return nc


def _get_program():
    if "nc" not in _prog_cache:
        _prog_cache["nc"] = _build_program()
    return _prog_cache["nc"]


def _host_small_math(l, context, g, W_context, W_graph, W_query, W_key):
    """concat (f32, matches reference op order) and v = norm-side vector."""
    lf = l.reshape(-1, D).astype(np.float32)
    cf = context.reshape(-1, D).astype(np.float32)
    gf = g.reshape(-1, D).astype(np.float32)
    Wc = np.asarray(W_context, np.float32)[0]   # (1,2)
    Wg = np.asarray(W_graph, np.float32)[0]     # (1,2)

    lc = np.concatenate([lf, cf], axis=0)       # (2, D)
    concat = (Wc @ lc).reshape(1, 1, D)         # f32, same as reference

    # device-side score vector in f64 for accuracy, cast to f32
    concat64 = concat.reshape(D).astype(np.float64)
    gctx = Wg[0, 0].astype(np.float64) * gf.reshape(D).astype(np.float64) \
        + Wg[0, 1].astype(np.float64) * concat64
    Q = gctx @ np.asarray(W_query, np.float64)          # (D,)
    v = np.asarray(W_key, np.float64) @ Q               # (D,)
    return concat, v.astype(np.float32)


def kernel(q, l, context, g, mask, is_random, random_net,
           W_context, W_graph, W_query, W_key):
    q = np.asarray(q)
    mask = np.asarray(mask)
    concat, v = _host_small_math(
        l, context, g, W_context, W_graph, W_query, W_key
    )

    # ---- shard-local constant inputs -------------------------------------
    qpad = np.zeros((N_PAD, D), dtype=np.float32)
    qpad[:N] = q.reshape(N, D).astype(np.float32)

    mask_valid = np.zeros(N_PAD, dtype=bool)
    mask_valid[:N] = mask.reshape(N) > 0

    madd_all = np.where(mask_valid, 0.0, MASK_NEG).astype(np.float32)
    mbin_all = mask_valid.astype(np.float32)
    idx_local = np.tile(
        (np.arange(NT, dtype=np.float32)[:, None] * P
         + np.arange(P, dtype=np.float32)[None, :]).reshape(NS),
        N_CORES,
    )

    def shard_pt(arr, c):
        # [NS] -> [P, NT] with [p, t] = node t*128+p of shard c
        return np.ascontiguousarray(
            arr[c * NS : (c + 1) * NS].reshape(NT, P).T
        )

    vb = v.reshape(1, D)
    in_maps = []
    for c in range(N_CORES):
        in_maps.append({
            "qs": qpad[c * NS : (c + 1) * NS],
            "vb": vb,
            "madd": shard_pt(madd_all, c),
            "mbin": shard_pt(mbin_all, c),
            "idxc": shard_pt(idx_local, c),
        })

    # ---- run on 8 NeuronCores --------------------------------------------
    nc = _get_program()
    res = run_bass_kernel_spmd(nc, in_maps, core_ids=list(range(N_CORES)))
    _prog_cache["last_results"] = res
    stats = np.stack([res.results[c]["stats"] for c in range(N_CORES)])
    # stats: [8, 128, 3] = (pmax, psumexp, pidx_local)

    # ---- host combine (O(1024)) ------------------------------------------
    pmax = stats[:, :, 0].astype(np.float64)
    psum = stats[:, :, 1].astype(np.float64)
    pidx = stats[:, :, 2]

    total = psum.sum()
    allmax = pmax.max()
    cand = np.argwhere(pmax == allmax)
    node = min(
        int(c) * NS + int(pidx[c, p]) for c, p in cand
    )

    if int(np.asarray(is_random)):
        idx = int(np.asarray(random_net).reshape(-1)[0])
        # score of the chosen node, computed exactly like the device path
        s = float(qpad[idx].astype(np.float64) @ v.astype(np.float64))
        c_val = 10.0 * math.tanh(NORM * s)
        if not mask_valid[idx]:
            attn = 0.0
            log_attn = -np.inf
        else:
            attn = math.exp(c_val) / total
            log_attn = c_val - math.log(total)
        max_indx = np.asarray(random_net).reshape(1, 1).astype(np.int32)
    else:
        idx = node
        attn = math.exp(allmax) / total
        log_attn = allmax - math.log(total)
        max_indx = np.array([[idx]], dtype=np.int32)

    q_max = q.reshape(N, D)[idx].reshape(1, 1, D).astype(np.float32)
    attn_max = np.array([[attn]], dtype=np.float32)
    log_attn_max = np.array([[log_attn]], dtype=np.float32)
    mask_copy = mask.reshape(1, N).astype(np.int32)

    return (q_max, attn_max, log_attn_max, concat, mask_copy, max_indx)


# revision 14
# speedup vs baseline: 1.3119x; 1.3119x over previous
"""Trainium2 Bass kernel for nn_MultiHeadDecoder (sparse attention decoder).

Math (reference, B=1, N=50000, D=512):
    concat    = W_context[0] @ [l; context]                  (1, D)
    g_context = W_graph[0]   @ [g; concat]                   (1, D)
    Q         = g_context @ W_query                          (1, D)
    K         = q @ W_key                                    (N, D)
    compat    = 10 * tanh(norm * Q @ K^T), masked -> -inf    (N,)
    outputs: q[argmax], softmax[argmax], log_softmax[argmax], concat, mask, argmax

Key algebraic optimization: scores = (q @ W_key) @ Q^T == q @ (W_key @ Q^T).
W_key @ Q^T is a tiny (D,D)@(D,) matvec done on host, so the device never
materializes K -- it streams q once and does a 50000x512 matvec + tanh +
masked softmax reductions. This is DMA-bound, not GEMM-bound.

Sharding: node dimension split across 8 cores, 6272 nodes/core (49 tiles of
128). Each core returns a per-partition (max, sum-exp, argmax) triple
([128,3] f32); the host does the final 1024-way combine (exact, tiny).
"""

import math

import numpy as np

import concourse.bass as bass
import concourse.tile as tile
from concourse import bacc, mybir
from concourse.bass_utils import run_bass_kernel_spmd

N_CORES = 8
N = 50000
D = 512
P = 128                      # SBUF partitions
NT = 49                      # node tiles per core
NS = P * NT                  # 6272 nodes per core
G = 7                        # node tiles per DMA chunk
N_PAD = N_CORES * NS         # 50176
NORM = 1.0 / math.sqrt(D)
MASK_NEG = -1000.0           # additive mask; scores are in [-10, 10]
BIG_IDX = 1.0e30
V_SCALE = 256.0              # keep v well inside fp16 normal range

_prog_cache = {}


def _build_program():
    """One SPMD Bass program: scores for a 6272-node shard + reductions.

    q and v stream in fp16 (halves HBM traffic, 2x DVE mode); the dot
    product accumulates in f32, everything downstream is f32.
    """
    f32 = mybir.dt.float32
    f16 = mybir.dt.float16
    nc = bacc.Bacc("TRN2", target_bir_lowering=False)

    qs = nc.dram_tensor("qs", [NS, D], f16, kind="ExternalInput")
    vb = nc.dram_tensor("vb", [1, D], f16, kind="ExternalInput")
    madd = nc.dram_tensor("madd", [P, NT], f32, kind="ExternalInput")
    mbin = nc.dram_tensor("mbin", [P, NT], f32, kind="ExternalInput")
    idxc = nc.dram_tensor("idxc", [P, NT], f32, kind="ExternalInput")
    stats = nc.dram_tensor("stats", [P, 3], f32, kind="ExternalOutput")

    # [NS, D] viewed as [P, NT, D]: column t of partition p is node t*128+p
    qsv = qs[:].rearrange("(t p) d -> p t d", p=P)

    with tile.TileContext(nc) as tc:
        with (
            tc.tile_pool(name="const", bufs=1) as constp,
            tc.tile_pool(name="qp", bufs=3) as qp,
            tc.tile_pool(name="work", bufs=2) as work,
            tc.tile_pool(name="acc", bufs=1) as accp,
        ):
            vbt = constp.tile([P, D], f16)
            nc.sync.dma_start(out=vbt[:], in_=vb[:].to_broadcast([P, D]))
            maddt = constp.tile([P, NT], f32)
            nc.sync.dma_start(out=maddt[:], in_=madd[:])
            mbint = constp.tile([P, NT], f32)
            nc.sync.dma_start(out=mbint[:], in_=mbin[:])
            idxt = constp.tile([P, NT], f32)
            nc.sync.dma_start(out=idxt[:], in_=idxc[:])
            bigt = constp.tile([P, NT], f32)
            nc.vector.memset(bigt[:], BIG_IDX)

            scores = accp.tile([P, NT], f32)

            for o in range(NT // G):
                qt = qp.tile([P, G, D], f16)
                nc.sync.dma_start(out=qt[:], in_=qsv[:, o * G : (o + 1) * G, :])
                for gi in range(G):
                    t = o * G + gi
                    prod = work.tile([P, D], f16)
                    # out = (q * 1.0) * v, accum_out = row-sum(out) = q . v
                    # (tensor_tensor_reduce dies at runtime on this stack;
                    # scalar_tensor_tensor's fused sum works)
                    nc.vector.scalar_tensor_tensor(
                        out=prod[:],
                        in0=qt[:, gi, :],
                        scalar=1.0,
                        in1=vbt[:],
                        op0=mybir.AluOpType.mult,
                        op1=mybir.AluOpType.mult,
                        accum_out=scores[:, t : t + 1],
                    )

            # tanh_t = tanh(norm * s);  cm = 10*tanh_t + madd (masked score)
            tanh_t = accp.tile([P, NT], f32)
            nc.scalar.activation(
                out=tanh_t[:], in_=scores[:],
                func=mybir.ActivationFunctionType.Tanh,
                scale=float(NORM / V_SCALE),
            )
            cm = accp.tile([P, NT], f32)
            nc.vector.scalar_tensor_tensor(
                out=cm[:], in0=tanh_t[:], scalar=10.0, in1=maddt[:],
                op0=mybir.AluOpType.mult, op1=mybir.AluOpType.add,
            )

            st = accp.tile([P, 3], f32)
            # per-partition max of masked scores
            nc.vector.reduce_max(
                out=st[:, 0:1], in_=cm[:], axis=mybir.AxisListType.X
            )
            # e = exp(10*tanh_t); masked sum via e * mbin with fused row-sum
            e_t = accp.tile([P, NT], f32)
            nc.scalar.activation(
                out=e_t[:], in_=tanh_t[:],
                func=mybir.ActivationFunctionType.Exp, scale=10.0,
            )
            escr = accp.tile([P, NT], f32)
            nc.vector.scalar_tensor_tensor(
                out=escr[:], in0=e_t[:], scalar=1.0, in1=mbint[:],
                op0=mybir.AluOpType.mult, op1=mybir.AluOpType.mult,
                accum_out=st[:, 1:2],
            )
            # argmax: first (lowest local index) column hitting the row max
            # (CopyPredicated requires an integer mask dtype)
            iseq = accp.tile([P, NT], mybir.dt.int32)
            nc.vector.tensor_scalar(
                out=iseq[:], in0=cm[:], scalar1=st[:, 0:1], scalar2=None,
                op0=mybir.AluOpType.is_ge,
            )
            idxsel = accp.tile([P, NT], f32)
            nc.vector.select(idxsel[:], iseq[:], idxt[:], bigt[:])
            nc.vector.tensor_reduce(
                out=st[:, 2:3], in_=idxsel[:],
                op=mybir.AluOpType.min, axis=mybir.AxisListType.X,
            )

            nc.sync.dma_start(out=stats[:], in_=st[:])

    nc.compile()
    return nc


def _get_program():
    if "nc" not in _prog_cache:
        _prog_cache["nc"] = _build_program()
    return _prog_cache["nc"]


def _host_small_math(l, context, g, W_context, W_graph, W_query, W_key):
    """concat (f32, matches reference op order) and v = norm-side vector."""
    lf = l.reshape(-1, D).astype(np.float32)
    cf = context.reshape(-1, D).astype(np.float32)
    gf = g.reshape(-1, D).astype(np.float32)
    Wc = np.asarray(W_context, np.float32)[0]   # (1,2)
    Wg = np.asarray(W_graph, np.float32)[0]     # (1,2)

    lc = np.concatenate([lf, cf], axis=0)       # (2, D)
    concat = (Wc @ lc).reshape(1, 1, D)         # f32, same as reference

    # device-side score vector in f64 for accuracy, cast to f32
    concat64 = concat.reshape(D).astype(np.float64)
    gctx = Wg[0, 0].astype(np.float64) * gf.reshape(D).astype(np.float64) \
        + Wg[0, 1].astype(np.float64) * concat64
    Q = gctx @ np.asarray(W_query, np.float64)          # (D,)
    v = np.asarray(W_key, np.float64) @ Q               # (D,)
    return concat, v.astype(np.float32)


def kernel(q, l, context, g, mask, is_random, random_net,
           W_context, W_graph, W_query, W_key):
    q = np.asarray(q)
    mask = np.asarray(mask)
    concat, v = _host_small_math(
        l, context, g, W_context, W_graph, W_query, W_key
    )

    # ---- shard-local constant inputs -------------------------------------
    qpad = np.zeros((N_PAD, D), dtype=np.float16)
    qpad[:N] = q.reshape(N, D).astype(np.float16)

    mask_valid = np.zeros(N_PAD, dtype=bool)
    mask_valid[:N] = mask.reshape(N) > 0

    madd_all = np.where(mask_valid, 0.0, MASK_NEG).astype(np.float32)
    mbin_all = mask_valid.astype(np.float32)
    idx_local = np.tile(
        (np.arange(NT, dtype=np.float32)[:, None] * P
         + np.arange(P, dtype=np.float32)[None, :]).reshape(NS),
        N_CORES,
    )

    def shard_pt(arr, c):
        # [NS] -> [P, NT] with [p, t] = node t*128+p of shard c
        return np.ascontiguousarray(
            arr[c * NS : (c + 1) * NS].reshape(NT, P).T
        )

    vb = (v * V_SCALE).astype(np.float16).reshape(1, D)
    in_maps = []
    for c in range(N_CORES):
        in_maps.append({
            "qs": qpad[c * NS : (c + 1) * NS],
            "vb": vb,
            "madd": shard_pt(madd_all, c),
            "mbin": shard_pt(mbin_all, c),
            "idxc": shard_pt(idx_local, c),
        })

    # ---- run on 8 NeuronCores --------------------------------------------
    nc = _get_program()
    res = run_bass_kernel_spmd(nc, in_maps, core_ids=list(range(N_CORES)))
    _prog_cache["last_results"] = res
    stats = np.stack([res.results[c]["stats"] for c in range(N_CORES)])
    # stats: [8, 128, 3] = (pmax, psumexp, pidx_local)

    # ---- host combine (O(1024)) ------------------------------------------
    pmax = stats[:, :, 0].astype(np.float64)
    psum = stats[:, :, 1].astype(np.float64)
    pidx = stats[:, :, 2]

    total = psum.sum()
    allmax = pmax.max()
    cand = np.argwhere(pmax == allmax)
    node = min(
        int(c) * NS + int(pidx[c, p]) for c, p in cand
    )

    q64 = q.reshape(N, D).astype(np.float64)
    v64 = v.astype(np.float64)

    def exact_score(i):
        return 10.0 * math.tanh(NORM * float(q64[i] @ v64))

    if int(np.asarray(is_random)):
        idx = int(np.asarray(random_net).reshape(-1)[0])
        c_val = exact_score(idx)
        if not mask_valid[idx]:
            attn = 0.0
            log_attn = -np.inf
        else:
            attn = math.exp(c_val) / total
            log_attn = c_val - math.log(total)
        max_indx = np.asarray(random_net).reshape(1, 1).astype(np.int32)
    else:
        idx = node
        # refine: exact argmax score on host; swap its term inside the
        # device-accumulated sum-of-exp (kills the fp16 error on the
        # numerator; the denominator residual is a softmax-weighted
        # average of independent fp16 errors, ~4e-5)
        c_exact = exact_score(idx)
        total = total - math.exp(allmax) + math.exp(c_exact)
        attn = math.exp(c_exact) / total
        log_attn = c_exact - math.log(total)
        max_indx = np.array([[idx]], dtype=np.int32)

    q_max = q.reshape(N, D)[idx].reshape(1, 1, D).astype(np.float32)
    attn_max = np.array([[attn]], dtype=np.float32)
    log_attn_max = np.array([[log_attn]], dtype=np.float32)
    mask_copy = mask.reshape(1, N).astype(np.int32)

    return (q_max, attn_max, log_attn_max, concat, mask_copy, max_indx)


# revision 19
# speedup vs baseline: 1.4318x; 1.0914x over previous
"""Trainium2 Bass kernel for nn_MultiHeadDecoder (sparse attention decoder).

Math (reference, B=1, N=50000, D=512):
    concat    = W_context[0] @ [l; context]                  (1, D)
    g_context = W_graph[0]   @ [g; concat]                   (1, D)
    Q         = g_context @ W_query                          (1, D)
    K         = q @ W_key                                    (N, D)
    compat    = 10 * tanh(norm * Q @ K^T), masked -> -inf    (N,)
    outputs: q[argmax], softmax[argmax], log_softmax[argmax], concat, mask, argmax

Key algebraic optimization: scores = (q @ W_key) @ Q^T == q @ (W_key @ Q^T).
W_key @ Q^T is a tiny (D,D)@(D,) matvec done on host, so the device never
materializes K -- it streams q once and does a 50000x512 matvec + tanh +
masked softmax reductions. This is DMA-bound, not GEMM-bound.

Sharding: node dimension split across 8 cores, 6272 nodes/core (49 tiles of
128). Each core returns a per-partition (max, sum-exp, argmax) triple
([128,3] f32); the host does the final 1024-way combine (exact, tiny).
"""

import math

import numpy as np

import concourse.bass as bass
import concourse.tile as tile
from concourse import bacc, mybir
from concourse.bass_utils import run_bass_kernel_spmd

N_CORES = 8
N = 50000
D = 512
P = 128                      # SBUF partitions
NT = 49                      # node tiles per core
NS = P * NT                  # 6272 nodes per core
G = 7                        # node tiles per DMA chunk
N_PAD = N_CORES * NS         # 50176
NORM = 1.0 / math.sqrt(D)
MASK_NEG = -1000.0           # additive mask; scores are in [-10, 10]
BIG_IDX = 1.0e30
V_SCALE = 256.0              # keep v well inside fp16 normal range

_prog_cache = {}


KC = D // P                  # 4 contraction chunks of 128 dims


def _build_program():
    """One SPMD Bass program: scores for a 6272-node shard + reductions.

    q streams in fp16, pre-transposed on host to dim-major [D, NS] so each
    DMA moves long contiguous runs per partition. The dot product runs on
    the TensorEngine: for each 128-node tile t and 128-dim chunk k,
    matmul(psum[:, t], lhsT=qT[k][:, t*128:(t+1)*128], rhs=v[k]) accumulates
    scores directly in [128 nodes, NT] PSUM layout. Everything downstream
    (tanh, masked softmax stats, argmax) is f32.
    """
    f32 = mybir.dt.float32
    f16 = mybir.dt.float16
    nc = bacc.Bacc("TRN2", target_bir_lowering=False)

    qst = nc.dram_tensor("qst", [D, NS], f16, kind="ExternalInput")
    vt = nc.dram_tensor("vt", [P, KC], f16, kind="ExternalInput")
    madd = nc.dram_tensor("madd", [P, NT], f32, kind="ExternalInput")
    mbin = nc.dram_tensor("mbin", [P, NT], f32, kind="ExternalInput")
    idxc = nc.dram_tensor("idxc", [P, NT], f32, kind="ExternalInput")
    stats = nc.dram_tensor("stats", [P, 3], f32, kind="ExternalOutput")

    with tile.TileContext(nc) as tc:
        with (
            tc.tile_pool(name="const", bufs=1) as constp,
            tc.tile_pool(name="qp", bufs=2) as qp,
            tc.tile_pool(name="acc", bufs=1) as accp,
            tc.tile_pool(name="ps", bufs=1, space="PSUM") as psp,
        ):
            vtt = constp.tile([P, KC], f16)
            nc.sync.dma_start(out=vtt[:], in_=vt[:])
            maddt = constp.tile([P, NT], f32)
            nc.sync.dma_start(out=maddt[:], in_=madd[:])
            mbint = constp.tile([P, NT], f32)
            nc.sync.dma_start(out=mbint[:], in_=mbin[:])
            idxt = constp.tile([P, NT], f32)
            nc.sync.dma_start(out=idxt[:], in_=idxc[:])
            bigt = constp.tile([P, NT], f32)
            nc.vector.memset(bigt[:], BIG_IDX)

            # node-tile blocks of G: DMA the 4 dim-chunk slices of a block,
            # then per column run its 4 accumulating matmuls back-to-back
            # (PSUM zero-region groups must not interleave within a bank)
            ps = psp.tile([P, NT], f32)
            for b in range(NT // G):
                cks = []
                for k in range(KC):
                    ck = qp.tile([P, G * P], f16, tag=f"ck{k}")
                    nc.sync.dma_start(
                        out=ck[:],
                        in_=qst[k * P : (k + 1) * P, b * G * P : (b + 1) * G * P],
                    )
                    cks.append(ck)
                for g in range(G):
                    t = b * G + g
                    for k in range(KC):
                        nc.tensor.matmul(
                            ps[:, t : t + 1],
                            cks[k][:, g * P : (g + 1) * P],
                            vtt[:, k : k + 1],
                            start=(k == 0),
                            stop=(k == KC - 1),
                        )

            scores = accp.tile([P, NT], f32)
            nc.scalar.copy(out=scores[:], in_=ps[:])

            # tanh_t = tanh(norm * s);  cm = 10*tanh_t + madd (masked score)
            tanh_t = accp.tile([P, NT], f32)
            nc.scalar.activation(
                out=tanh_t[:], in_=scores[:],
                func=mybir.ActivationFunctionType.Tanh,
                scale=float(NORM / V_SCALE),
            )
            cm = accp.tile([P, NT], f32)
            nc.vector.scalar_tensor_tensor(
                out=cm[:], in0=tanh_t[:], scalar=10.0, in1=maddt[:],
                op0=mybir.AluOpType.mult, op1=mybir.AluOpType.add,
            )

            st = accp.tile([P, 3], f32)
            # per-partition max of masked scores
            nc.vector.reduce_max(
                out=st[:, 0:1], in_=cm[:], axis=mybir.AxisListType.X
            )
            # e = exp(10*tanh_t); masked sum via e * mbin with fused row-sum
            e_t = accp.tile([P, NT], f32)
            nc.scalar.activation(
                out=e_t[:], in_=tanh_t[:],
                func=mybir.ActivationFunctionType.Exp, scale=10.0,
            )
            escr = accp.tile([P, NT], f32)
            nc.vector.scalar_tensor_tensor(
                out=escr[:], in0=e_t[:], scalar=1.0, in1=mbint[:],
                op0=mybir.AluOpType.mult, op1=mybir.AluOpType.mult,
                accum_out=st[:, 1:2],
            )
            # argmax: first (lowest local index) column hitting the row max
            # (CopyPredicated requires an integer mask dtype)
            iseq = accp.tile([P, NT], mybir.dt.int32)
            nc.vector.tensor_scalar(
                out=iseq[:], in0=cm[:], scalar1=st[:, 0:1], scalar2=None,
                op0=mybir.AluOpType.is_ge,
            )
            idxsel = accp.tile([P, NT], f32)
            nc.vector.select(idxsel[:], iseq[:], idxt[:], bigt[:])
            nc.vector.tensor_reduce(
                out=st[:, 2:3], in_=idxsel[:],
                op=mybir.AluOpType.min, axis=mybir.AxisListType.X,
            )

            nc.sync.dma_start(out=stats[:], in_=st[:])

    nc.compile()
    return nc


def _get_program():
    if "nc" not in _prog_cache:
        _prog_cache["nc"] = _build_program()
    return _prog_cache["nc"]


def _host_small_math(l, context, g, W_context, W_graph, W_query, W_key):
    """concat (f32, matches reference op order) and v = norm-side vector."""
    lf = l.reshape(-1, D).astype(np.float32)
    cf = context.reshape(-1, D).astype(np.float32)
    gf = g.reshape(-1, D).astype(np.float32)
    Wc = np.asarray(W_context, np.float32)[0]   # (1,2)
    Wg = np.asarray(W_graph, np.float32)[0]     # (1,2)

    lc = np.concatenate([lf, cf], axis=0)       # (2, D)
    concat = (Wc @ lc).reshape(1, 1, D)         # f32, same as reference

    # device-side score vector in f64 for accuracy, cast to f32
    concat64 = concat.reshape(D).astype(np.float64)
    gctx = Wg[0, 0].astype(np.float64) * gf.reshape(D).astype(np.float64) \
        + Wg[0, 1].astype(np.float64) * concat64
    Q = gctx @ np.asarray(W_query, np.float64)          # (D,)
    v = np.asarray(W_key, np.float64) @ Q               # (D,)
    return concat, v.astype(np.float32)


def kernel(q, l, context, g, mask, is_random, random_net,
           W_context, W_graph, W_query, W_key):
    q = np.asarray(q)
    mask = np.asarray(mask)
    concat, v = _host_small_math(
        l, context, g, W_context, W_graph, W_query, W_key
    )

    # ---- shard-local constant inputs -------------------------------------
    # dim-major fp16: qpadT[d, n] = q[n, d]; per-core slice [D, NS] is a
    # row-contiguous block -> long per-partition DMA runs on device
    qpadT = np.zeros((D, N_PAD), dtype=np.float16)
    qpadT[:, :N] = q.reshape(N, D).astype(np.float16).T

    mask_valid = np.zeros(N_PAD, dtype=bool)
    mask_valid[:N] = mask.reshape(N) > 0

    madd_all = np.where(mask_valid, 0.0, MASK_NEG).astype(np.float32)
    mbin_all = mask_valid.astype(np.float32)
    idx_local = np.tile(
        (np.arange(NT, dtype=np.float32)[:, None] * P
         + np.arange(P, dtype=np.float32)[None, :]).reshape(NS),
        N_CORES,
    )

    def shard_pt(arr, c):
        # [NS] -> [P, NT] with [p, t] = node t*128+p of shard c
        return np.ascontiguousarray(
            arr[c * NS : (c + 1) * NS].reshape(NT, P).T
        )

    vt = np.ascontiguousarray(
        (v * V_SCALE).astype(np.float16).reshape(KC, P).T
    )  # vt[p, k] = v_scaled[k*128 + p]
    in_maps = []
    for c in range(N_CORES):
        in_maps.append({
            "qst": np.ascontiguousarray(qpadT[:, c * NS : (c + 1) * NS]),
            "vt": vt,
            "madd": shard_pt(madd_all, c),
            "mbin": shard_pt(mbin_all, c),
            "idxc": shard_pt(idx_local, c),
        })

    # ---- run on 8 NeuronCores --------------------------------------------
    nc = _get_program()
    res = run_bass_kernel_spmd(nc, in_maps, core_ids=list(range(N_CORES)))
    _prog_cache["last_results"] = res
    stats = np.stack([res.results[c]["stats"] for c in range(N_CORES)])
    # stats: [8, 128, 3] = (pmax, psumexp, pidx_local)

    # ---- host combine (O(1024)) ------------------------------------------
    pmax = stats[:, :, 0].astype(np.float64)
    psum = stats[:, :, 1].astype(np.float64)
    pidx = stats[:, :, 2]

    total = psum.sum()
    allmax = pmax.max()
    cand = np.argwhere(pmax == allmax)
    node = min(
        int(c) * NS + int(pidx[c, p]) for c, p in cand
    )

    q64 = q.reshape(N, D).astype(np.float64)
    v64 = v.astype(np.float64)

    def exact_score(i):
        return 10.0 * math.tanh(NORM * float(q64[i] @ v64))

    if int(np.asarray(is_random)):
        idx = int(np.asarray(random_net).reshape(-1)[0])
        c_val = exact_score(idx)
        if not mask_valid[idx]:
            attn = 0.0
            log_attn = -np.inf
        else:
            attn = math.exp(c_val) / total
            log_attn = c_val - math.log(total)
        max_indx = np.asarray(random_net).reshape(1, 1).astype(np.int32)
    else:
        idx = node
        # refine: exact argmax score on host; swap its term inside the
        # device-accumulated sum-of-exp (kills the fp16 error on the
        # numerator; the denominator residual is a softmax-weighted
        # average of independent fp16 errors, ~4e-5)
        c_exact = exact_score(idx)
        total = total - math.exp(allmax) + math.exp(c_exact)
        attn = math.exp(c_exact) / total
        log_attn = c_exact - math.log(total)
        max_indx = np.array([[idx]], dtype=np.int32)

    q_max = q.reshape(N, D)[idx].reshape(1, 1, D).astype(np.float32)
    attn_max = np.array([[attn]], dtype=np.float32)
    log_attn_max = np.array([[log_attn]], dtype=np.float32)
    mask_copy = mask.reshape(1, N).astype(np.int32)

    return (q_max, attn_max, log_attn_max, concat, mask_copy, max_indx)


# revision 25
# speedup vs baseline: 1.7009x; 1.1879x over previous
"""Trainium2 Bass kernel for nn_MultiHeadDecoder (sparse attention decoder).

Math (reference, B=1, N=50000, D=512):
    concat    = W_context[0] @ [l; context]                  (1, D)
    g_context = W_graph[0]   @ [g; concat]                   (1, D)
    Q         = g_context @ W_query                          (1, D)
    K         = q @ W_key                                    (N, D)
    compat    = 10 * tanh(norm * Q @ K^T), masked -> -inf    (N,)
    outputs: q[argmax], softmax[argmax], log_softmax[argmax], concat, mask, argmax

Key algebraic optimization: scores = (q @ W_key) @ Q^T == q @ (W_key @ Q^T).
W_key @ Q^T is a tiny (D,D)@(D,) matvec done on host, so the device never
materializes K -- it streams q once and does a 50000x512 matvec + tanh +
masked softmax reductions. This makes the kernel HBM-bound, not GEMM-bound.

Device mapping (per core, 6272-node shard, fp16 streaming):
  - q is host-transposed to dim-major [D, NS] fp16; 4 dim-chunks of 128
    rows DMA in as [128, NS] tiles (12.5 KB contiguous per partition).
  - TensorE: per chunk k, v_k [128,1] is the stationary operand; 13
    matmuls of N=512 nodes write partial scores [1, 512] into row j of
    PSUM bank k. DVE sums the 4 banks -> scores [13, 512] f32.
  - ACT/DVE epilogue: tanh, masked max / sum-exp / first-argmax per
    partition row -> stats [13, 3] back to host.
  - Host: O(100) combine across 8 cores, exact argmax-score refinement.
"""

import math

import numpy as np

import concourse.bass as bass
import concourse.tile as tile
from concourse import bacc, mybir
from concourse.bass_utils import run_bass_kernel_spmd

N_CORES = 8
N = 50000
D = 512
P = 128                      # SBUF partitions
NT = 49                      # 128-node tiles per core
NS = P * NT                  # 6272 nodes per core shard
N_PAD = N_CORES * NS         # 50176
KC = D // P                  # 4 contraction chunks of 128 dims
NORM = 1.0 / math.sqrt(D)
MASK_NEG = -1000.0           # additive mask; real scores are in [-10, 10]
BIG_IDX = 1.0e30
V_SCALE = 256.0              # keep v well inside fp16 normal range

_prog_cache = {}


def _build_program():
    f32 = mybir.dt.float32
    f16 = mybir.dt.float16
    nc = bacc.Bacc("TRN2", target_bir_lowering=False)

    qst = nc.dram_tensor("qst", [D, NS], f16, kind="ExternalInput")
    vt = nc.dram_tensor("vt", [P, KC], f16, kind="ExternalInput")
    madd = nc.dram_tensor("madd", [P, NT], f32, kind="ExternalInput")
    mbin = nc.dram_tensor("mbin", [P, NT], f32, kind="ExternalInput")
    idxc = nc.dram_tensor("idxc", [P, NT], f32, kind="ExternalInput")
    stats = nc.dram_tensor("stats", [P, 3], f32, kind="ExternalOutput")

    with tile.TileContext(nc) as tc:
        with (
            tc.tile_pool(name="const", bufs=1) as constp,
            tc.tile_pool(name="qp", bufs=1) as qp,
            tc.tile_pool(name="acc", bufs=1) as accp,
            tc.tile_pool(name="ps", bufs=1, space="PSUM") as psp,
        ):
            # q-chunk DMAs first: they pace the kernel. 12.5 KB contiguous
            # per partition per chunk -> near-peak HBM efficiency.
            cks = []
            for k in range(KC):
                ck = qp.tile([P, NS], f16, tag=f"ck{k}")
                nc.sync.dma_start(out=ck[:], in_=qst[k * P : (k + 1) * P, :])
                cks.append(ck)

            vtt = constp.tile([P, KC], f16)
            nc.sync.dma_start(out=vtt[:], in_=vt[:])
            maddt = constp.tile([P, NT], f32)
            nc.sync.dma_start(out=maddt[:], in_=madd[:])
            mbint = constp.tile([P, NT], f32)
            nc.sync.dma_start(out=mbint[:], in_=mbin[:])
            idxt = constp.tile([P, NT], f32)
            nc.sync.dma_start(out=idxt[:], in_=idxc[:])
            bigt = constp.tile([P, NT], f32)
            nc.vector.memset(bigt[:], BIG_IDX)

            # partial scores: PSUM bank k holds chunk k's [128 nodes, NT]
            # column dots; every (k, t) location is written by exactly one
            # matmul (start=stop=True) so groups never interleave in a bank
            pss = []
            for k in range(KC):
                ps = psp.tile([P, NT], f32, tag=f"ps{k}")
                pss.append(ps)
            for k in range(KC):
                for t in range(NT):
                    nc.tensor.matmul(
                        pss[k][:, t : t + 1],
                        cks[k][:, t * P : (t + 1) * P],
                        vtt[:, k : k + 1],
                        start=True,
                        stop=True,
                    )

            # combine the 4 chunk banks -> scores [P, NT] f32 in SBUF
            # (an op may read at most one PSUM input: copy, then 3 adds)
            scores = accp.tile([P, NT], f32)
            nc.scalar.copy(out=scores[:], in_=pss[0][:])
            nc.vector.tensor_add(scores[:], scores[:], pss[1][:])
            nc.vector.tensor_add(scores[:], scores[:], pss[2][:])
            nc.vector.tensor_add(scores[:], scores[:], pss[3][:])

            # tanh_t = tanh(norm * s);  cm = 10*tanh_t + madd (masked score)
            tanh_t = accp.tile([P, NT], f32)
            nc.scalar.activation(
                out=tanh_t[:], in_=scores[:],
                func=mybir.ActivationFunctionType.Tanh,
                scale=float(NORM / V_SCALE),
            )
            cm = accp.tile([P, NT], f32)
            nc.vector.scalar_tensor_tensor(
                out=cm[:], in0=tanh_t[:], scalar=10.0, in1=maddt[:],
                op0=mybir.AluOpType.mult, op1=mybir.AluOpType.add,
            )

            st = accp.tile([P, 3], f32)
            nc.vector.reduce_max(
                out=st[:, 0:1], in_=cm[:], axis=mybir.AxisListType.X
            )
            # e = exp(10*tanh_t); masked row-sum via (e*1.0)*mbin fused accum
            e_t = accp.tile([P, NT], f32)
            nc.scalar.activation(
                out=e_t[:], in_=tanh_t[:],
                func=mybir.ActivationFunctionType.Exp, scale=10.0,
            )
            escr = accp.tile([P, NT], f32)
            nc.vector.scalar_tensor_tensor(
                out=escr[:], in0=e_t[:], scalar=1.0, in1=mbint[:],
                op0=mybir.AluOpType.mult, op1=mybir.AluOpType.mult,
                accum_out=st[:, 1:2],
            )
            # argmax: first (lowest local index) column hitting the row max
            iseq = accp.tile([P, NT], mybir.dt.int32)
            nc.vector.tensor_scalar(
                out=iseq[:], in0=cm[:], scalar1=st[:, 0:1], scalar2=None,
                op0=mybir.AluOpType.is_ge,
            )
            idxsel = accp.tile([P, NT], f32)
            nc.vector.select(idxsel[:], iseq[:], idxt[:], bigt[:])
            nc.vector.tensor_reduce(
                out=st[:, 2:3], in_=idxsel[:],
                op=mybir.AluOpType.min, axis=mybir.AxisListType.X,
            )

            nc.sync.dma_start(out=stats[:], in_=st[:])

    nc.compile()
    return nc


def _get_program():
    if "nc" not in _prog_cache:
        _prog_cache["nc"] = _build_program()
    return _prog_cache["nc"]


def _host_small_math(l, context, g, W_context, W_graph, W_query, W_key):
    """concat (f32, matches reference op order) and v: scores = q @ v."""
    lf = l.reshape(-1, D).astype(np.float32)
    cf = context.reshape(-1, D).astype(np.float32)
    gf = g.reshape(-1, D).astype(np.float32)
    Wc = np.asarray(W_context, np.float32)[0]   # (1,2)
    Wg = np.asarray(W_graph, np.float32)[0]     # (1,2)

    lc = np.concatenate([lf, cf], axis=0)       # (2, D)
    concat = (Wc @ lc).reshape(1, 1, D)         # f32, same as reference

    concat64 = concat.reshape(D).astype(np.float64)
    gctx = Wg[0, 0].astype(np.float64) * gf.reshape(D).astype(np.float64) \
        + Wg[0, 1].astype(np.float64) * concat64
    Q = gctx @ np.asarray(W_query, np.float64)          # (D,)
    v = np.asarray(W_key, np.float64) @ Q               # (D,)
    return concat, v


def kernel(q, l, context, g, mask, is_random, random_net,
           W_context, W_graph, W_query, W_key):
    q = np.asarray(q)
    mask = np.asarray(mask)
    concat, v = _host_small_math(
        l, context, g, W_context, W_graph, W_query, W_key
    )

    # ---- shard inputs -----------------------------------------------------
    # dim-major fp16 q: per-core [D, NS] block -> long per-partition DMA runs
    qpadT = np.zeros((D, N_PAD), dtype=np.float16)
    qpadT[:, :N] = q.reshape(N, D).astype(np.float16).T

    mask_valid = np.zeros(N_PAD, dtype=bool)
    mask_valid[:N] = mask.reshape(N) > 0

    vt = np.ascontiguousarray(
        (v * V_SCALE).astype(np.float16).reshape(KC, P).T
    )  # vt[p, k] = v_scaled[k*128 + p]

    def shard_pt(arr, c):
        # [NS] shard -> [P, NT] with [p, t] = local node t*128+p
        return np.ascontiguousarray(
            arr[c * NS : (c + 1) * NS].reshape(NT, P).T
        )

    madd_all = np.where(mask_valid, 0.0, MASK_NEG).astype(np.float32)
    mbin_all = mask_valid.astype(np.float32)
    idx_all = np.arange(N_PAD, dtype=np.float32) % NS  # local index

    in_maps = []
    for c in range(N_CORES):
        in_maps.append({
            "qst": np.ascontiguousarray(qpadT[:, c * NS : (c + 1) * NS]),
            "vt": vt,
            "madd": shard_pt(madd_all, c),
            "mbin": shard_pt(mbin_all, c),
            "idxc": shard_pt(idx_all, c),
        })

    # ---- run on 8 NeuronCores --------------------------------------------
    nc = _get_program()
    res = run_bass_kernel_spmd(nc, in_maps, core_ids=list(range(N_CORES)))
    _prog_cache["last_results"] = res
    stats = np.stack([res.results[c]["stats"] for c in range(N_CORES)])
    # stats: [8, P, 3] = (row max, row sum-exp, row argmax local idx)

    # ---- host combine (O(100)) -------------------------------------------
    pmax = stats[:, :, 0].astype(np.float64)
    psum = stats[:, :, 1].astype(np.float64)
    pidx = stats[:, :, 2]

    total = psum.sum()
    allmax = pmax.max()
    cand = np.argwhere(pmax == allmax)
    node = min(int(c) * NS + int(pidx[c, r]) for c, r in cand)

    q64 = q.reshape(N, D).astype(np.float64)
    v64 = v.astype(np.float64)

    def exact_score(i):
        return 10.0 * math.tanh(NORM * float(q64[i] @ v64))

    if int(np.asarray(is_random)):
        idx = int(np.asarray(random_net).reshape(-1)[0])
        c_val = exact_score(idx)
        if not mask_valid[idx]:
            attn = 0.0
            log_attn = -np.inf
        else:
            attn = math.exp(c_val) / total
            log_attn = c_val - math.log(total)
        max_indx = np.asarray(random_net).reshape(1, 1).astype(np.int32)
    else:
        idx = node
        # refine: exact argmax score on host; swap its term inside the
        # device-accumulated sum-of-exp (kills the fp16 error on the
        # numerator; the denominator residual is a softmax-weighted
        # average of independent fp16 errors, ~4e-5)
        c_exact = exact_score(idx)
        total = total - math.exp(allmax) + math.exp(c_exact)
        attn = math.exp(c_exact) / total
        log_attn = c_exact - math.log(total)
        max_indx = np.array([[idx]], dtype=np.int32)

    q_max = q.reshape(N, D)[idx].reshape(1, 1, D).astype(np.float32)
    attn_max = np.array([[attn]], dtype=np.float32)
    log_attn_max = np.array([[log_attn]], dtype=np.float32)
    mask_copy = mask.reshape(1, N).astype(np.int32)

    return (q_max, attn_max, log_attn_max, concat, mask_copy, max_indx)


# revision 27
# speedup vs baseline: 1.7945x; 1.0551x over previous
"""Trainium2 Bass kernel for nn_MultiHeadDecoder (sparse attention decoder).

Math (reference, B=1, N=50000, D=512):
    concat    = W_context[0] @ [l; context]                  (1, D)
    g_context = W_graph[0]   @ [g; concat]                   (1, D)
    Q         = g_context @ W_query                          (1, D)
    K         = q @ W_key                                    (N, D)
    compat    = 10 * tanh(norm * Q @ K^T), masked -> -inf    (N,)
    outputs: q[argmax], softmax[argmax], log_softmax[argmax], concat, mask, argmax

Key algebraic optimization: scores = (q @ W_key) @ Q^T == q @ (W_key @ Q^T).
W_key @ Q^T is a tiny (D,D)@(D,) matvec done on host, so the device never
materializes K -- it streams q once and does a 50000x512 matvec + tanh +
masked softmax reductions. This makes the kernel HBM-bound, not GEMM-bound.

Device mapping (per core, 6272-node shard, fp16 streaming):
  - q is host-transposed to dim-major [D, NS] fp16; 4 dim-chunks of 128
    rows DMA in as [128, NS] tiles (12.5 KB contiguous per partition).
  - TensorE: per chunk k, v_k [128,1] is the stationary operand; 13
    matmuls of N=512 nodes write partial scores [1, 512] into row j of
    PSUM bank k. DVE sums the 4 banks -> scores [13, 512] f32.
  - ACT/DVE epilogue: tanh, masked max / sum-exp / first-argmax per
    partition row -> stats [13, 3] back to host.
  - Host: O(100) combine across 8 cores, exact argmax-score refinement.
"""

import math

import numpy as np

import concourse.bass as bass
import concourse.tile as tile
from concourse import bacc, mybir
from concourse.bass_utils import run_bass_kernel_spmd

N_CORES = 8
N = 50000
D = 512
P = 128                      # SBUF partitions
NT = 49                      # 128-node tiles per core
NS = P * NT                  # 6272 nodes per core shard
N_PAD = N_CORES * NS         # 50176
KC = D // P                  # 4 contraction chunks of 128 dims
NORM = 1.0 / math.sqrt(D)
MASK_NEG = -1000.0           # additive mask; real scores are in [-10, 10]
BIG_IDX = 1.0e30
V_SCALE = 256.0              # keep v well inside fp16 normal range

_prog_cache = {}


def _build_program():
    f32 = mybir.dt.float32
    f16 = mybir.dt.float16
    nc = bacc.Bacc("TRN2", target_bir_lowering=False)

    qst = nc.dram_tensor("qst", [D, NS], f16, kind="ExternalInput")
    vt = nc.dram_tensor("vt", [P, KC], f16, kind="ExternalInput")
    madd = nc.dram_tensor("madd", [P, NT], f32, kind="ExternalInput")
    mbin = nc.dram_tensor("mbin", [P, NT], f32, kind="ExternalInput")
    idxc = nc.dram_tensor("idxc", [P, NT], f32, kind="ExternalInput")
    stats = nc.dram_tensor("stats", [P, 3], f32, kind="ExternalOutput")

    with tile.TileContext(nc) as tc:
        with (
            tc.tile_pool(name="const", bufs=1) as constp,
            tc.tile_pool(name="qp", bufs=1) as qp,
            tc.tile_pool(name="acc", bufs=1) as accp,
            tc.tile_pool(name="ps", bufs=1, space="PSUM") as psp,
        ):
            vtt = constp.tile([P, KC], f16)
            nc.sync.dma_start(out=vtt[:], in_=vt[:])

            # q-chunk DMAs next: they pace the kernel. Each 128-dim chunk
            # is split into 4 node-quarters so the PE can start on a
            # quarter as soon as it lands (3-3.3 KB contiguous runs per
            # partition keep HBM efficiency near peak).
            qbounds = [0, 13, 25, 37, NT]
            ckq = {}
            for k in range(KC):
                for qi in range(4):
                    lo, hi = qbounds[qi], qbounds[qi + 1]
                    tle = qp.tile([P, (hi - lo) * P], f16, tag=f"ck{k}q{qi}")
                    nc.sync.dma_start(
                        out=tle[:],
                        in_=qst[k * P : (k + 1) * P, lo * P : hi * P],
                    )
                    ckq[(k, qi)] = tle
            maddt = constp.tile([P, NT], f32)
            nc.sync.dma_start(out=maddt[:], in_=madd[:])
            mbint = constp.tile([P, NT], f32)
            nc.sync.dma_start(out=mbint[:], in_=mbin[:])
            idxt = constp.tile([P, NT], f32)
            nc.sync.dma_start(out=idxt[:], in_=idxc[:])
            bigt = constp.tile([P, NT], f32)
            nc.vector.memset(bigt[:], BIG_IDX)

            # partial scores: PSUM bank k holds chunk k's [128 nodes, NT]
            # column dots; every (k, t) location is written by exactly one
            # matmul (start=stop=True) so groups never interleave in a bank
            pss = []
            for k in range(KC):
                ps = psp.tile([P, NT], f32, tag=f"ps{k}")
                pss.append(ps)
            for k in range(KC):
                for qi in range(4):
                    lo, hi = qbounds[qi], qbounds[qi + 1]
                    tle = ckq[(k, qi)]
                    for t in range(lo, hi):
                        nc.tensor.matmul(
                            pss[k][:, t : t + 1],
                            tle[:, (t - lo) * P : (t - lo + 1) * P],
                            vtt[:, k : k + 1],
                            start=True,
                            stop=True,
                        )

            # combine the 4 chunk banks -> scores [P, NT] f32 in SBUF
            # (an op may read at most one PSUM input: copy, then 3 adds)
            scores = accp.tile([P, NT], f32)
            nc.scalar.copy(out=scores[:], in_=pss[0][:])
            nc.vector.tensor_add(scores[:], scores[:], pss[1][:])
            nc.vector.tensor_add(scores[:], scores[:], pss[2][:])
            nc.vector.tensor_add(scores[:], scores[:], pss[3][:])

            # tanh_t = tanh(norm * s);  cm = 10*tanh_t + madd (masked score)
            tanh_t = accp.tile([P, NT], f32)
            nc.scalar.activation(
                out=tanh_t[:], in_=scores[:],
                func=mybir.ActivationFunctionType.Tanh,
                scale=float(NORM / V_SCALE),
            )
            cm = accp.tile([P, NT], f32)
            nc.vector.scalar_tensor_tensor(
                out=cm[:], in0=tanh_t[:], scalar=10.0, in1=maddt[:],
                op0=mybir.AluOpType.mult, op1=mybir.AluOpType.add,
            )

            st = accp.tile([P, 3], f32)
            nc.vector.reduce_max(
                out=st[:, 0:1], in_=cm[:], axis=mybir.AxisListType.X
            )
            # e = exp(10*tanh_t); masked row-sum via (e*1.0)*mbin fused accum
            e_t = accp.tile([P, NT], f32)
            nc.scalar.activation(
                out=e_t[:], in_=tanh_t[:],
                func=mybir.ActivationFunctionType.Exp, scale=10.0,
            )
            escr = accp.tile([P, NT], f32)
            nc.vector.scalar_tensor_tensor(
                out=escr[:], in0=e_t[:], scalar=1.0, in1=mbint[:],
                op0=mybir.AluOpType.mult, op1=mybir.AluOpType.mult,
                accum_out=st[:, 1:2],
            )
            # argmax: first (lowest local index) column hitting the row max
            iseq = accp.tile([P, NT], mybir.dt.int32)
            nc.vector.tensor_scalar(
                out=iseq[:], in0=cm[:], scalar1=st[:, 0:1], scalar2=None,
                op0=mybir.AluOpType.is_ge,
            )
            idxsel = accp.tile([P, NT], f32)
            nc.vector.select(idxsel[:], iseq[:], idxt[:], bigt[:])
            nc.vector.tensor_reduce(
                out=st[:, 2:3], in_=idxsel[:],
                op=mybir.AluOpType.min, axis=mybir.AxisListType.X,
            )

            nc.sync.dma_start(out=stats[:], in_=st[:])

    nc.compile()
    return nc


def _get_program():
    if "nc" not in _prog_cache:
        _prog_cache["nc"] = _build_program()
    return _prog_cache["nc"]


def _host_small_math(l, context, g, W_context, W_graph, W_query, W_key):
    """concat (f32, matches reference op order) and v: scores = q @ v."""
    lf = l.reshape(-1, D).astype(np.float32)
    cf = context.reshape(-1, D).astype(np.float32)
    gf = g.reshape(-1, D).astype(np.float32)
    Wc = np.asarray(W_context, np.float32)[0]   # (1,2)
    Wg = np.asarray(W_graph, np.float32)[0]     # (1,2)

    lc = np.concatenate([lf, cf], axis=0)       # (2, D)
    concat = (Wc @ lc).reshape(1, 1, D)         # f32, same as reference

    concat64 = concat.reshape(D).astype(np.float64)
    gctx = Wg[0, 0].astype(np.float64) * gf.reshape(D).astype(np.float64) \
        + Wg[0, 1].astype(np.float64) * concat64
    Q = gctx @ np.asarray(W_query, np.float64)          # (D,)
    v = np.asarray(W_key, np.float64) @ Q               # (D,)
    return concat, v


def kernel(q, l, context, g, mask, is_random, random_net,
           W_context, W_graph, W_query, W_key):
    q = np.asarray(q)
    mask = np.asarray(mask)
    concat, v = _host_small_math(
        l, context, g, W_context, W_graph, W_query, W_key
    )

    # ---- shard inputs -----------------------------------------------------
    # dim-major fp16 q: per-core [D, NS] block -> long per-partition DMA runs
    qpadT = np.zeros((D, N_PAD), dtype=np.float16)
    qpadT[:, :N] = q.reshape(N, D).astype(np.float16).T

    mask_valid = np.zeros(N_PAD, dtype=bool)
    mask_valid[:N] = mask.reshape(N) > 0

    vt = np.ascontiguousarray(
        (v * V_SCALE).astype(np.float16).reshape(KC, P).T
    )  # vt[p, k] = v_scaled[k*128 + p]

    def shard_pt(arr, c):
        # [NS] shard -> [P, NT] with [p, t] = local node t*128+p
        return np.ascontiguousarray(
            arr[c * NS : (c + 1) * NS].reshape(NT, P).T
        )

    madd_all = np.where(mask_valid, 0.0, MASK_NEG).astype(np.float32)
    mbin_all = mask_valid.astype(np.float32)
    idx_all = np.arange(N_PAD, dtype=np.float32) % NS  # local index

    in_maps = []
    for c in range(N_CORES):
        in_maps.append({
            "qst": np.ascontiguousarray(qpadT[:, c * NS : (c + 1) * NS]),
            "vt": vt,
            "madd": shard_pt(madd_all, c),
            "mbin": shard_pt(mbin_all, c),
            "idxc": shard_pt(idx_all, c),
        })

    # ---- run on 8 NeuronCores --------------------------------------------
    nc = _get_program()
    res = run_bass_kernel_spmd(nc, in_maps, core_ids=list(range(N_CORES)))
    _prog_cache["last_results"] = res
    stats = np.stack([res.results[c]["stats"] for c in range(N_CORES)])
    # stats: [8, P, 3] = (row max, row sum-exp, row argmax local idx)

    # ---- host combine (O(100)) -------------------------------------------
    pmax = stats[:, :, 0].astype(np.float64)
    psum = stats[:, :, 1].astype(np.float64)
    pidx = stats[:, :, 2]

    total = psum.sum()
    allmax = pmax.max()
    cand = np.argwhere(pmax == allmax)
    node = min(int(c) * NS + int(pidx[c, r]) for c, r in cand)

    q64 = q.reshape(N, D).astype(np.float64)
    v64 = v.astype(np.float64)

    def exact_score(i):
        return 10.0 * math.tanh(NORM * float(q64[i] @ v64))

    if int(np.asarray(is_random)):
        idx = int(np.asarray(random_net).reshape(-1)[0])
        c_val = exact_score(idx)
        if not mask_valid[idx]:
            attn = 0.0
            log_attn = -np.inf
        else:
            attn = math.exp(c_val) / total
            log_attn = c_val - math.log(total)
        max_indx = np.asarray(random_net).reshape(1, 1).astype(np.int32)
    else:
        idx = node
        # refine: exact argmax score on host; swap its term inside the
        # device-accumulated sum-of-exp (kills the fp16 error on the
        # numerator; the denominator residual is a softmax-weighted
        # average of independent fp16 errors, ~4e-5)
        c_exact = exact_score(idx)
        total = total - math.exp(allmax) + math.exp(c_exact)
        attn = math.exp(c_exact) / total
        log_attn = c_exact - math.log(total)
        max_indx = np.array([[idx]], dtype=np.int32)

    q_max = q.reshape(N, D)[idx].reshape(1, 1, D).astype(np.float32)
    attn_max = np.array([[attn]], dtype=np.float32)
    log_attn_max = np.array([[log_attn]], dtype=np.float32)
    mask_copy = mask.reshape(1, N).astype(np.int32)

    return (q_max, attn_max, log_attn_max, concat, mask_copy, max_indx)


# revision 29
# speedup vs baseline: 1.8213x; 1.0149x over previous
"""Trainium2 Bass kernel for nn_MultiHeadDecoder (sparse attention decoder).

Math (reference, B=1, N=50000, D=512):
    concat    = W_context[0] @ [l; context]                  (1, D)
    g_context = W_graph[0]   @ [g; concat]                   (1, D)
    Q         = g_context @ W_query                          (1, D)
    K         = q @ W_key                                    (N, D)
    compat    = 10 * tanh(norm * Q @ K^T), masked -> -inf    (N,)
    outputs: q[argmax], softmax[argmax], log_softmax[argmax], concat, mask, argmax

Key algebraic optimization: scores = (q @ W_key) @ Q^T == q @ (W_key @ Q^T).
W_key @ Q^T is a tiny (D,D)@(D,) matvec done on host, so the device never
materializes K -- it streams q once and does a 50000x512 matvec + tanh +
masked softmax reductions. This makes the kernel HBM-bound, not GEMM-bound.

Device mapping (per core, 6272-node shard, fp16 streaming):
  - q is host-transposed to dim-major [D, NS] fp16; 4 dim-chunks of 128
    rows DMA in as [128, NS] tiles (12.5 KB contiguous per partition).
  - TensorE: per chunk k, v_k [128,1] is the stationary operand; 13
    matmuls of N=512 nodes write partial scores [1, 512] into row j of
    PSUM bank k. DVE sums the 4 banks -> scores [13, 512] f32.
  - ACT/DVE epilogue: tanh, masked max / sum-exp / first-argmax per
    partition row -> stats [13, 3] back to host.
  - Host: O(100) combine across 8 cores, exact argmax-score refinement.
"""

import math

import numpy as np

import concourse.bass as bass
import concourse.tile as tile
from concourse import bacc, mybir
from concourse.bass_utils import run_bass_kernel_spmd

N_CORES = 8
N = 50000
D = 512
P = 128                      # SBUF partitions
NT = 49                      # 128-node tiles per core
NS = P * NT                  # 6272 nodes per core shard
N_PAD = N_CORES * NS         # 50176
KC = D // P                  # 4 contraction chunks of 128 dims
NORM = 1.0 / math.sqrt(D)
MASK_NEG = -1000.0           # additive mask; real scores are in [-10, 10]
BIG_IDX = 1.0e30
V_SCALE = 256.0              # keep v well inside fp16 normal range

_prog_cache = {}


def _build_program():
    f32 = mybir.dt.float32
    f16 = mybir.dt.float16
    nc = bacc.Bacc("TRN2", target_bir_lowering=False)

    qst = nc.dram_tensor("qst", [D, NS], f16, kind="ExternalInput")
    vt = nc.dram_tensor("vt", [P, KC], f16, kind="ExternalInput")
    madd = nc.dram_tensor("madd", [P, NT], f32, kind="ExternalInput")
    mbin = nc.dram_tensor("mbin", [P, NT], f32, kind="ExternalInput")
    idxc = nc.dram_tensor("idxc", [P, NT], f32, kind="ExternalInput")
    stats = nc.dram_tensor("stats", [P, 3], f32, kind="ExternalOutput")

    with tile.TileContext(nc) as tc:
        with (
            tc.tile_pool(name="const", bufs=1) as constp,
            tc.tile_pool(name="qp", bufs=1) as qp,
            tc.tile_pool(name="acc", bufs=1) as accp,
            tc.tile_pool(name="ps", bufs=1, space="PSUM") as psp,
        ):
            vtt = constp.tile([P, KC], f16)
            nc.sync.dma_start(out=vtt[:], in_=vt[:])

            # q-chunk DMAs next: they pace the kernel. Each 128-dim chunk
            # is split into 4 node-quarters so the PE can start on a
            # quarter as soon as it lands (3-3.3 KB contiguous runs per
            # partition keep HBM efficiency near peak).
            qbounds = [0, 4, 13, 25, 37, NT]
            ckq = {}
            for k in range(KC):
                for qi in range(len(qbounds) - 1):
                    lo, hi = qbounds[qi], qbounds[qi + 1]
                    tle = qp.tile([P, (hi - lo) * P], f16, tag=f"ck{k}q{qi}")
                    nc.sync.dma_start(
                        out=tle[:],
                        in_=qst[k * P : (k + 1) * P, lo * P : hi * P],
                    )
                    ckq[(k, qi)] = tle
            maddt = constp.tile([P, NT], f32)
            nc.sync.dma_start(out=maddt[:], in_=madd[:])
            mbint = constp.tile([P, NT], f32)
            nc.sync.dma_start(out=mbint[:], in_=mbin[:])
            idxt = constp.tile([P, NT], f32)
            nc.sync.dma_start(out=idxt[:], in_=idxc[:])
            bigt = constp.tile([P, NT], f32)
            nc.vector.memset(bigt[:], BIG_IDX)

            # partial scores: PSUM bank k holds chunk k's [128 nodes, NT]
            # column dots; every (k, t) location is written by exactly one
            # matmul (start=stop=True) so groups never interleave in a bank
            pss = []
            for k in range(KC):
                ps = psp.tile([P, NT], f32, tag=f"ps{k}")
                pss.append(ps)
            for k in range(KC):
                for qi in range(len(qbounds) - 1):
                    lo, hi = qbounds[qi], qbounds[qi + 1]
                    tle = ckq[(k, qi)]
                    for t in range(lo, hi):
                        nc.tensor.matmul(
                            pss[k][:, t : t + 1],
                            tle[:, (t - lo) * P : (t - lo + 1) * P],
                            vtt[:, k : k + 1],
                            start=True,
                            stop=True,
                        )

            # combine the 4 chunk banks -> scores [P, NT] f32 in SBUF
            # (an op may read at most one PSUM input: copy, then 3 adds)
            scores = accp.tile([P, NT], f32)
            nc.scalar.copy(out=scores[:], in_=pss[0][:])
            nc.vector.tensor_add(scores[:], scores[:], pss[1][:])
            nc.vector.tensor_add(scores[:], scores[:], pss[2][:])
            nc.vector.tensor_add(scores[:], scores[:], pss[3][:])

            # tanh_t = tanh(norm * s);  cm = 10*tanh_t + madd (masked score)
            tanh_t = accp.tile([P, NT], f32)
            nc.scalar.activation(
                out=tanh_t[:], in_=scores[:],
                func=mybir.ActivationFunctionType.Tanh,
                scale=float(NORM / V_SCALE),
            )
            cm = accp.tile([P, NT], f32)
            nc.vector.scalar_tensor_tensor(
                out=cm[:], in0=tanh_t[:], scalar=10.0, in1=maddt[:],
                op0=mybir.AluOpType.mult, op1=mybir.AluOpType.add,
            )

            st = accp.tile([P, 3], f32)
            nc.vector.reduce_max(
                out=st[:, 0:1], in_=cm[:], axis=mybir.AxisListType.X
            )
            # e = exp(10*tanh_t); masked row-sum via (e*1.0)*mbin fused accum
            e_t = accp.tile([P, NT], f32)
            nc.scalar.activation(
                out=e_t[:], in_=tanh_t[:],
                func=mybir.ActivationFunctionType.Exp, scale=10.0,
            )
            escr = accp.tile([P, NT], f32)
            nc.vector.scalar_tensor_tensor(
                out=escr[:], in0=e_t[:], scalar=1.0, in1=mbint[:],
                op0=mybir.AluOpType.mult, op1=mybir.AluOpType.mult,
                accum_out=st[:, 1:2],
            )
            # argmax: first (lowest local index) column hitting the row max
            iseq = accp.tile([P, NT], mybir.dt.int32)
            nc.vector.tensor_scalar(
                out=iseq[:], in0=cm[:], scalar1=st[:, 0:1], scalar2=None,
                op0=mybir.AluOpType.is_ge,
            )
            idxsel = accp.tile([P, NT], f32)
            nc.vector.select(idxsel[:], iseq[:], idxt[:], bigt[:])
            nc.vector.tensor_reduce(
                out=st[:, 2:3], in_=idxsel[:],
                op=mybir.AluOpType.min, axis=mybir.AxisListType.X,
            )

            nc.sync.dma_start(out=stats[:], in_=st[:])

    nc.compile()
    return nc


def _get_program():
    if "nc" not in _prog_cache:
        _prog_cache["nc"] = _build_program()
    return _prog_cache["nc"]


def _host_small_math(l, context, g, W_context, W_graph, W_query, W_key):
    """concat (f32, matches reference op order) and v: scores = q @ v."""
    lf = l.reshape(-1, D).astype(np.float32)
    cf = context.reshape(-1, D).astype(np.float32)
    gf = g.reshape(-1, D).astype(np.float32)
    Wc = np.asarray(W_context, np.float32)[0]   # (1,2)
    Wg = np.asarray(W_graph, np.float32)[0]     # (1,2)

    lc = np.concatenate([lf, cf], axis=0)       # (2, D)
    concat = (Wc @ lc).reshape(1, 1, D)         # f32, same as reference

    concat64 = concat.reshape(D).astype(np.float64)
    gctx = Wg[0, 0].astype(np.float64) * gf.reshape(D).astype(np.float64) \
        + Wg[0, 1].astype(np.float64) * concat64
    Q = gctx @ np.asarray(W_query, np.float64)          # (D,)
    v = np.asarray(W_key, np.float64) @ Q               # (D,)
    return concat, v


def kernel(q, l, context, g, mask, is_random, random_net,
           W_context, W_graph, W_query, W_key):
    q = np.asarray(q)
    mask = np.asarray(mask)
    concat, v = _host_small_math(
        l, context, g, W_context, W_graph, W_query, W_key
    )

    # ---- shard inputs -----------------------------------------------------
    # dim-major fp16 q: per-core [D, NS] block -> long per-partition DMA runs
    qpadT = np.zeros((D, N_PAD), dtype=np.float16)
    qpadT[:, :N] = q.reshape(N, D).astype(np.float16).T

    mask_valid = np.zeros(N_PAD, dtype=bool)
    mask_valid[:N] = mask.reshape(N) > 0

    vt = np.ascontiguousarray(
        (v * V_SCALE).astype(np.float16).reshape(KC, P).T
    )  # vt[p, k] = v_scaled[k*128 + p]

    def shard_pt(arr, c):
        # [NS] shard -> [P, NT] with [p, t] = local node t*128+p
        return np.ascontiguousarray(
            arr[c * NS : (c + 1) * NS].reshape(NT, P).T
        )

    madd_all = np.where(mask_valid, 0.0, MASK_NEG).astype(np.float32)
    mbin_all = mask_valid.astype(np.float32)
    idx_all = np.arange(N_PAD, dtype=np.float32) % NS  # local index

    in_maps = []
    for c in range(N_CORES):
        in_maps.append({
            "qst": np.ascontiguousarray(qpadT[:, c * NS : (c + 1) * NS]),
            "vt": vt,
            "madd": shard_pt(madd_all, c),
            "mbin": shard_pt(mbin_all, c),
            "idxc": shard_pt(idx_all, c),
        })

    # ---- run on 8 NeuronCores --------------------------------------------
    nc = _get_program()
    res = run_bass_kernel_spmd(nc, in_maps, core_ids=list(range(N_CORES)))
    _prog_cache["last_results"] = res
    stats = np.stack([res.results[c]["stats"] for c in range(N_CORES)])
    # stats: [8, P, 3] = (row max, row sum-exp, row argmax local idx)

    # ---- host combine (O(100)) -------------------------------------------
    pmax = stats[:, :, 0].astype(np.float64)
    psum = stats[:, :, 1].astype(np.float64)
    pidx = stats[:, :, 2]

    total = psum.sum()
    allmax = pmax.max()
    cand = np.argwhere(pmax == allmax)
    node = min(int(c) * NS + int(pidx[c, r]) for c, r in cand)

    q64 = q.reshape(N, D).astype(np.float64)
    v64 = v.astype(np.float64)

    def exact_score(i):
        return 10.0 * math.tanh(NORM * float(q64[i] @ v64))

    if int(np.asarray(is_random)):
        idx = int(np.asarray(random_net).reshape(-1)[0])
        c_val = exact_score(idx)
        if not mask_valid[idx]:
            attn = 0.0
            log_attn = -np.inf
        else:
            attn = math.exp(c_val) / total
            log_attn = c_val - math.log(total)
        max_indx = np.asarray(random_net).reshape(1, 1).astype(np.int32)
    else:
        idx = node
        # refine: exact argmax score on host; swap its term inside the
        # device-accumulated sum-of-exp (kills the fp16 error on the
        # numerator; the denominator residual is a softmax-weighted
        # average of independent fp16 errors, ~4e-5)
        c_exact = exact_score(idx)
        total = total - math.exp(allmax) + math.exp(c_exact)
        attn = math.exp(c_exact) / total
        log_attn = c_exact - math.log(total)
        max_indx = np.array([[idx]], dtype=np.int32)

    q_max = q.reshape(N, D)[idx].reshape(1, 1, D).astype(np.float32)
    attn_max = np.array([[attn]], dtype=np.float32)
    log_attn_max = np.array([[log_attn]], dtype=np.float32)
    mask_copy = mask.reshape(1, N).astype(np.int32)

    return (q_max, attn_max, log_attn_max, concat, mask_copy, max_indx)
